# revision 1
# baseline (speedup 1.0000x reference)
"""ChunkStickyRouter Trainium2 kernel.

Strategy (8 NeuronCores, data-parallel over tokens):
  - Flatten [B=4, S=4096] -> 16384 tokens, shard 2048 tokens per core.
  - Per core, run the 3-layer router MLP in feature-major layout
    (features on SBUF partitions, tokens on the free dim) so each layer's
    output feeds the next without transposes:
        h1T[H, T]  = relu(w1.T @ xT + b1)
        h2T[H2, T] = relu(w2.T @ h1T + b2)
        lT [E, T]  = w3.T @ h2T
    then reduce lT over 128-token chunks -> per-chunk logit sums [E, 16].
  - Matmuls run as float32r (tf32: 10-bit mantissa, fp32 accumulate), 4x the
    fp32 rate on the PE array. x / weights are RTNE-rounded to tf32 on the
    host; h1/h2 are rounded by the ACT engine writing float32r tiles.
  - Host gathers the tiny [8, E, 16] chunk sums, adds b3/128-mean, and runs
    the sequential hysteresis scan + one-hot + scalar stats in numpy
    (microseconds of work; the scan is inherently sequential and tiny).
"""

import numpy as np

# Problem shapes (hardcoded per contract)
B, S, D, H, E = 4, 4096, 2048, 1024, 8
H2 = H // 2
CHUNK = 128
TAU = 0.7
NCORES = 8
T_LOC = (B * S) // NCORES          # 2048 tokens per core
C_LOC = T_LOC // CHUNK             # 16 chunks per core
P = 128
TT = 512                           # token tile (matmul moving free dim)
NT = T_LOC // TT                   # 4 token tiles per core
KD = D // P                        # 16 k-subtiles for layer 1
M1 = H // P                        # 8 output tiles for layer 1
K2 = H // P                        # 8 k-subtiles for layer 2
M2 = H2 // P                       # 4 output tiles for layer 2
K3 = H2 // P                       # 4 k-subtiles for layer 3
CPT = TT // CHUNK                  # 4 chunks per token tile

_CACHE = {}


def _round_tf32(x: np.ndarray) -> np.ndarray:
    """fp32 -> tf32 (10 explicit mantissa bits) round-to-nearest-even."""
    u = np.ascontiguousarray(x, dtype=np.float32).view(np.uint32).astype(np.uint64)
    bias = ((u >> 13) & 1) + 0xFFF
    u = (u + bias) & ~np.uint64(0x1FFF)
    return u.astype(np.uint32).view(np.float32)


def _build_module():
    import concourse.bass as bass  # noqa: F401
    import concourse.mybir as mybir
    import concourse.tile as tile
    from concourse import bacc

    F32 = mybir.dt.float32
    F32R = mybir.dt.float32r
    ts = bass.ts

    nc = bacc.Bacc("TRN2", target_bir_lowering=False, debug=False)

    xt = nc.dram_tensor("xt", [NT, P, KD, TT], F32R, kind="ExternalInput")
    w1 = nc.dram_tensor("w1", [D, H], F32R, kind="ExternalInput")
    w2 = nc.dram_tensor("w2", [H, H2], F32R, kind="ExternalInput")
    w3 = nc.dram_tensor("w3", [H2, E], F32R, kind="ExternalInput")
    b1 = nc.dram_tensor("b1", [H], F32, kind="ExternalInput")
    b2 = nc.dram_tensor("b2", [H2], F32, kind="ExternalInput")
    csum_out = nc.dram_tensor("csum", [E, C_LOC], F32, kind="ExternalOutput")

    w1r = w1.rearrange("(kd p) h -> p kd h", p=P)
    w2r = w2.rearrange("(kh p) h -> p kh h", p=P)
    w3r = w3.rearrange("(kh p) e -> p kh e", p=P)
    b1r = b1.rearrange("(m p) -> p m", p=P)
    b2r = b2.rearrange("(m p) -> p m", p=P)

    Relu = mybir.ActivationFunctionType.Relu

    with tile.TileContext(nc) as tc:
        with (
            tc.tile_pool(name="wp", bufs=1) as wp,
            tc.tile_pool(name="xp", bufs=2) as xp,
            tc.tile_pool(name="hp", bufs=1) as hp,
            tc.tile_pool(name="op", bufs=1) as op,
            tc.tile_pool(name="ps", bufs=4, space="PSUM") as ps,
            tc.tile_pool(name="ps3", bufs=2, space="PSUM") as ps3,
        ):
            w1s = wp.tile([P, KD, H], F32R)
            w2s = wp.tile([P, K2, H2], F32R)
            w3s = wp.tile([P, K3, E], F32R)
            b1s = wp.tile([P, M1], F32)
            b2s = wp.tile([P, M2], F32)
            nc.sync.dma_start(w1s[:], w1r)
            nc.sync.dma_start(w2s[:], w2r)
            nc.sync.dma_start(w3s[:], w3r)
            nc.sync.dma_start(b1s[:], b1r)
            nc.sync.dma_start(b2s[:], b2r)

            csums = op.tile([E, C_LOC], F32)

            for nt in range(NT):
                x_t = xp.tile([P, KD, TT], F32R, tag="x")
                nc.sync.dma_start(x_t[:], xt[nt])

                h1 = hp.tile([P, M1, TT], F32R, tag="h1")
                for m in range(M1):
                    pt = ps.tile([P, TT], F32, tag="acc")
                    for k in range(KD):
                        nc.tensor.matmul(pt[:], w1s[:, k, ts(m, P)], x_t[:, k, :],
                                         start=(k == 0), stop=(k == KD - 1))
                    nc.scalar.activation(h1[:, m, :], pt[:], Relu, bias=b1s[:, m : m + 1])

                h2 = hp.tile([P, M2, TT], F32R, tag="h2")
                for m in range(M2):
                    pt = ps.tile([P, TT], F32, tag="acc")
                    for k in range(K2):
                        nc.tensor.matmul(pt[:], w2s[:, k, ts(m, P)], h1[:, k, :],
                                         start=(k == 0), stop=(k == K2 - 1))
                    nc.scalar.activation(h2[:, m, :], pt[:], Relu, bias=b2s[:, m : m + 1])

                p3 = ps3.tile([E, TT], F32, tag="l3")
                for k in range(K3):
                    nc.tensor.matmul(p3[:], w3s[:, k, :], h2[:, k, :],
                                     start=(k == 0), stop=(k == K3 - 1))
                lg = op.tile([E, TT], F32, tag="lg")
                nc.scalar.copy(lg[:], p3[:])
                nc.vector.reduce_sum(csums[:, ts(nt, CPT)],
                                     lg.rearrange("p (c t) -> p c t", t=CHUNK),
                                     axis=mybir.AxisListType.X)

            nc.sync.dma_start(csum_out[:], csums[:])

    nc.compile()
    return nc


def _get_module():
    if "nc" not in _CACHE:
        _CACHE["nc"] = _build_module()
    return _CACHE["nc"]


def _device_chunk_sums(x, w1, w2, w3, b1, b2):
    """Run the sharded MLP on 8 NeuronCores; return chunk logit sums [B, C, E]
    (without the +b3 and /CHUNK, which the host applies)."""
    from concourse.bass_utils import run_bass_kernel_spmd

    nc = _get_module()

    w1r = _round_tf32(w1)
    w2r = _round_tf32(w2)
    w3r = _round_tf32(w3)
    b1c = np.ascontiguousarray(b1, dtype=np.float32)
    b2c = np.ascontiguousarray(b2, dtype=np.float32)

    flat = x.reshape(NCORES, T_LOC, D)
    in_maps = []
    for c in range(NCORES):
        # [T_LOC, D] -> [NT, P, KD, TT]: xt[nt, p, kd, tt] = x[nt*TT + tt, kd*P + p]
        xt = flat[c].reshape(NT, TT, KD, P).transpose(0, 3, 2, 1)
        in_maps.append({
            "xt": _round_tf32(xt),
            "w1": w1r, "w2": w2r, "w3": w3r, "b1": b1c, "b2": b2c,
        })

    res = run_bass_kernel_spmd(nc, in_maps, core_ids=list(range(NCORES)))
    csums = np.stack([r["csum"] for r in res.results])          # [8, E, C_LOC]
    return csums.transpose(0, 2, 1).reshape(B, S // CHUNK, E)   # [B, C, E]


def kernel(x, prev_expert_indices=None, w1=None, b1=None, w2=None, b2=None,
           w3=None, b3=None):
    x = np.ascontiguousarray(x, dtype=np.float32)
    C = S // CHUNK

    chunk_sums = _device_chunk_sums(x, w1, w2, w3, b1, b2)
    chunk_logits = (chunk_sums / np.float32(CHUNK)
                    + np.asarray(b3, np.float32)[None, None, :]).astype(np.float32)

    # Sequential hysteresis scan over chunks (mirrors the reference scan).
    top = np.argmax(chunk_logits, axis=-1).astype(np.int32)     # [B, C]
    prev = np.zeros(B, np.int32)
    experts = np.zeros((B, C), np.int32)
    flips = 0
    bidx = np.arange(B)
    for c in range(C):
        cur = chunk_logits[bidx, c, top[:, c]]
        pv = chunk_logits[bidx, c, prev]
        switch = (cur - pv) > np.float32(TAU)
        if c == 0:
            final = top[:, c]
        else:
            final = np.where(switch, top[:, c], prev)
            flips += int(switch.sum())
        experts[:, c] = final
        prev = final

    one_hot = np.zeros((B, C, E), np.float32)
    np.put_along_axis(one_hot, experts[..., None].astype(np.int64), 1.0, axis=2)
    routing_weights = np.broadcast_to(one_hot[:, :, None, :],
                                      (B, C, CHUNK, E)).reshape(B, S, E)
    routing_weights = np.ascontiguousarray(routing_weights)

    # Stats (fp32, mirroring the reference formulas)
    m = chunk_logits.max(-1, keepdims=True)
    ex = np.exp(chunk_logits - m, dtype=np.float32)
    probs = ex / ex.sum(-1, keepdims=True)
    gate_entropy = np.float32(-(probs * np.log(probs + np.float32(1e-8))).sum(-1).mean())
    expert_utilization = one_hot.mean((0, 1)).astype(np.float32)
    flip_rate = np.float32(flips) / np.float32(B * max(1, C - 1))
    routing_concentration = np.float32(np.linalg.norm(expert_utilization))

    return (routing_weights, experts, chunk_logits,
            np.float32(gate_entropy), expert_utilization,
            np.float32(flip_rate), np.float32(routing_concentration))


# revision 4
# speedup vs baseline: 638.0890x; 638.0890x over previous
"""ChunkStickyRouter Trainium2 kernel.

Strategy (8 NeuronCores, data-parallel over tokens):
  - Flatten [B=4, S=4096] -> 16384 tokens, shard 2048 tokens per core.
  - Per core, run the 3-layer router MLP in feature-major layout
    (features on SBUF partitions, tokens on the free dim) so each layer's
    output feeds the next without transposes:
        h1T[H, T]  = relu(w1.T @ xT + b1)
        h2T[H2, T] = relu(w2.T @ h1T + b2)
        lT [E, T]  = w3.T @ h2T
    then reduce lT over 128-token chunks -> per-chunk logit sums [E, 16].
  - Matmuls run as float32r (tf32: 10-bit mantissa, fp32 accumulate), 4x the
    fp32 rate on the PE array. x / weights are RTNE-rounded to tf32 on the
    host; h1/h2 are rounded by the ACT engine writing float32r tiles.
  - Host gathers the tiny [8, E, 16] chunk sums, adds b3/128-mean, and runs
    the sequential hysteresis scan + one-hot + scalar stats in numpy
    (microseconds of work; the scan is inherently sequential and tiny).
"""

import numpy as np

# Problem shapes (hardcoded per contract)
B, S, D, H, E = 4, 4096, 2048, 1024, 8
H2 = H // 2
CHUNK = 128
TAU = 0.7
NCORES = 8
T_LOC = (B * S) // NCORES          # 2048 tokens per core
C_LOC = T_LOC // CHUNK             # 16 chunks per core
P = 128
TT = 512                           # token tile (matmul moving free dim)
NT = T_LOC // TT                   # 4 token tiles per core
KD = D // P                        # 16 k-subtiles for layer 1
M1 = H // P                        # 8 output tiles for layer 1
K2 = H // P                        # 8 k-subtiles for layer 2
M2 = H2 // P                       # 4 output tiles for layer 2
K3 = H2 // P                       # 4 k-subtiles for layer 3
CPT = TT // CHUNK                  # 4 chunks per token tile

_CACHE = {}


def _round_tf32(x: np.ndarray) -> np.ndarray:
    """fp32 -> tf32 (10 explicit mantissa bits) round-to-nearest-even."""
    u = np.ascontiguousarray(x, dtype=np.float32).view(np.uint32).astype(np.uint64)
    bias = ((u >> 13) & 1) + 0xFFF
    u = (u + bias) & ~np.uint64(0x1FFF)
    return u.astype(np.uint32).view(np.float32)


def _build_module(passes=1):
    import concourse.bass as bass  # noqa: F401
    import concourse.mybir as mybir
    import concourse.tile as tile
    from concourse import bacc

    F32 = mybir.dt.float32
    F32R = mybir.dt.float32r
    ts = bass.ts

    nc = bacc.Bacc("TRN2", target_bir_lowering=False, debug=False)

    xt = nc.dram_tensor("xt", [NT, P, KD, TT], F32R, kind="ExternalInput")
    w1 = nc.dram_tensor("w1", [D, H], F32R, kind="ExternalInput")
    w2 = nc.dram_tensor("w2", [H, H2], F32R, kind="ExternalInput")
    w3 = nc.dram_tensor("w3", [H2, E], F32R, kind="ExternalInput")
    b1 = nc.dram_tensor("b1", [H], F32, kind="ExternalInput")
    b2 = nc.dram_tensor("b2", [H2], F32, kind="ExternalInput")
    csum_out = nc.dram_tensor("csum", [E, C_LOC], F32, kind="ExternalOutput")

    w1r = w1.rearrange("(kd p) h -> p kd h", p=P)
    w2r = w2.rearrange("(kh p) h -> p kh h", p=P)
    w3r = w3.rearrange("(kh p) e -> p kh e", p=P)
    b1r = b1.rearrange("(m p) -> p m", p=P)
    b2r = b2.rearrange("(m p) -> p m", p=P)

    Relu = mybir.ActivationFunctionType.Relu

    with tile.TileContext(nc) as tc:
        with (
            tc.tile_pool(name="wp", bufs=1) as wp,
            tc.tile_pool(name="xp", bufs=2) as xp,
            tc.tile_pool(name="hp", bufs=1) as hp,
            tc.tile_pool(name="op", bufs=1) as op,
            tc.tile_pool(name="ps", bufs=4, space="PSUM") as ps,
            tc.tile_pool(name="ps3", bufs=2, space="PSUM") as ps3,
        ):
            w1s = wp.tile([P, KD, H], F32R)
            w2s = wp.tile([P, K2, H2], F32R)
            w3s = wp.tile([P, K3, E], F32R)
            b1s = wp.tile([P, M1], F32)
            b2s = wp.tile([P, M2], F32)
            nc.sync.dma_start(w1s[:], w1r)
            nc.sync.dma_start(w2s[:], w2r)
            nc.sync.dma_start(w3s[:], w3r)
            nc.sync.dma_start(b1s[:], b1r)
            nc.sync.dma_start(b2s[:], b2r)

            csums = op.tile([E, C_LOC], F32)

            for _p in range(passes):
                for nt in range(NT):
                    x_t = xp.tile([P, KD, TT], F32R, tag="x")
                    nc.sync.dma_start(x_t[:], xt[nt])

                    h1 = hp.tile([P, M1, TT], F32R, tag="h1")
                    for m in range(M1):
                        pt = ps.tile([P, TT], F32, tag="acc")
                        for k in range(KD):
                            nc.tensor.matmul(pt[:], w1s[:, k, ts(m, P)], x_t[:, k, :],
                                             start=(k == 0), stop=(k == KD - 1))
                        nc.scalar.activation(h1[:, m, :], pt[:], Relu, bias=b1s[:, m : m + 1])

                    h2 = hp.tile([P, M2, TT], F32R, tag="h2")
                    for m in range(M2):
                        pt = ps.tile([P, TT], F32, tag="acc")
                        for k in range(K2):
                            nc.tensor.matmul(pt[:], w2s[:, k, ts(m, P)], h1[:, k, :],
                                             start=(k == 0), stop=(k == K2 - 1))
                        nc.scalar.activation(h2[:, m, :], pt[:], Relu, bias=b2s[:, m : m + 1])

                    p3 = ps3.tile([E, TT], F32, tag="l3")
                    for k in range(K3):
                        nc.tensor.matmul(p3[:], w3s[:, k, :], h2[:, k, :],
                                         start=(k == 0), stop=(k == K3 - 1))
                    lg = op.tile([E, TT], F32, tag="lg")
                    nc.scalar.copy(lg[:], p3[:])
                    nc.vector.reduce_sum(csums[:, ts(nt, CPT)],
                                         lg.rearrange("p (c t) -> p c t", t=CHUNK),
                                         axis=mybir.AxisListType.X)

            nc.sync.dma_start(csum_out[:], csums[:])

    nc.compile()
    return nc


def _get_module():
    if "nc" not in _CACHE:
        _CACHE["nc"] = _build_module()
    return _CACHE["nc"]


def _device_chunk_sums(x, w1, w2, w3, b1, b2):
    """Run the sharded MLP on 8 NeuronCores; return chunk logit sums [B, C, E]
    (without the +b3 and /CHUNK, which the host applies)."""
    from concourse.bass_utils import run_bass_kernel_spmd

    nc = _get_module()

    w1r = _round_tf32(w1)
    w2r = _round_tf32(w2)
    w3r = _round_tf32(w3)
    b1c = np.ascontiguousarray(b1, dtype=np.float32)
    b2c = np.ascontiguousarray(b2, dtype=np.float32)

    flat = x.reshape(NCORES, T_LOC, D)
    in_maps = []
    for c in range(NCORES):
        # [T_LOC, D] -> [NT, P, KD, TT]: xt[nt, p, kd, tt] = x[nt*TT + tt, kd*P + p]
        xt = flat[c].reshape(NT, TT, KD, P).transpose(0, 3, 2, 1)
        in_maps.append({
            "xt": _round_tf32(xt),
            "w1": w1r, "w2": w2r, "w3": w3r, "b1": b1c, "b2": b2c,
        })

    res = run_bass_kernel_spmd(nc, in_maps, core_ids=list(range(NCORES)))
    csums = np.stack([r["csum"] for r in res.results])          # [8, E, C_LOC]
    return csums.transpose(0, 2, 1).reshape(B, S // CHUNK, E)   # [B, C, E]


def kernel(x, prev_expert_indices=None, w1=None, b1=None, w2=None, b2=None,
           w3=None, b3=None):
    x = np.ascontiguousarray(x, dtype=np.float32)
    C = S // CHUNK

    chunk_sums = _device_chunk_sums(x, w1, w2, w3, b1, b2)
    chunk_logits = (chunk_sums / np.float32(CHUNK)
                    + np.asarray(b3, np.float32)[None, None, :]).astype(np.float32)

    # Sequential hysteresis scan over chunks (mirrors the reference scan).
    top = np.argmax(chunk_logits, axis=-1).astype(np.int32)     # [B, C]
    prev = np.zeros(B, np.int32)
    experts = np.zeros((B, C), np.int32)
    flips = 0
    bidx = np.arange(B)
    for c in range(C):
        cur = chunk_logits[bidx, c, top[:, c]]
        pv = chunk_logits[bidx, c, prev]
        switch = (cur - pv) > np.float32(TAU)
        if c == 0:
            final = top[:, c]
        else:
            final = np.where(switch, top[:, c], prev)
            flips += int(switch.sum())
        experts[:, c] = final
        prev = final

    one_hot = np.zeros((B, C, E), np.float32)
    np.put_along_axis(one_hot, experts[..., None].astype(np.int64), 1.0, axis=2)
    routing_weights = np.broadcast_to(one_hot[:, :, None, :],
                                      (B, C, CHUNK, E)).reshape(B, S, E)
    routing_weights = np.ascontiguousarray(routing_weights)

    # Stats (fp32, mirroring the reference formulas)
    m = chunk_logits.max(-1, keepdims=True)
    ex = np.exp(chunk_logits - m, dtype=np.float32)
    probs = ex / ex.sum(-1, keepdims=True)
    gate_entropy = np.float32(-(probs * np.log(probs + np.float32(1e-8))).sum(-1).mean())
    expert_utilization = one_hot.mean((0, 1)).astype(np.float32)
    flip_rate = np.float32(flips) / np.float32(B * max(1, C - 1))
    routing_concentration = np.float32(np.linalg.norm(expert_utilization))

    return (routing_weights, experts, chunk_logits,
            np.float32(gate_entropy), expert_utilization,
            np.float32(flip_rate), np.float32(routing_concentration))


# revision 8
# speedup vs baseline: 992.4332x; 1.5553x over previous
"""ChunkStickyRouter Trainium2 kernel.

Strategy (8 NeuronCores, data-parallel over tokens):
  - Flatten [B=4, S=4096] -> 16384 tokens, shard 2048 tokens per core.
  - Per core, run the 3-layer router MLP in feature-major layout
    (features on SBUF partitions, tokens on the free dim) so each layer's
    output feeds the next without transposes:
        h1T[H, T]  = relu(w1.T @ xT + b1)
        h2T[H2, T] = relu(w2.T @ h1T + b2)
        lT [E, T]  = w3.T @ h2T
    then reduce lT over 128-token chunks -> per-chunk logit sums [E, 16].
  - Matmuls run as float32r (tf32: 10-bit mantissa, fp32 accumulate), 4x the
    fp32 rate on the PE array. x / weights are RTNE-rounded to tf32 on the
    host; h1/h2 are rounded by the ACT engine writing float32r tiles.
  - Host gathers the tiny [8, E, 16] chunk sums, adds b3/128-mean, and runs
    the sequential hysteresis scan + one-hot + scalar stats in numpy
    (microseconds of work; the scan is inherently sequential and tiny).
"""

import numpy as np

# Problem shapes (hardcoded per contract)
B, S, D, H, E = 4, 4096, 2048, 1024, 8
H2 = H // 2
CHUNK = 128
TAU = 0.7
NCORES = 8
T_LOC = (B * S) // NCORES          # 2048 tokens per core
C_LOC = T_LOC // CHUNK             # 16 chunks per core
P = 128
TT = 512                           # token tile (matmul moving free dim)
NT = T_LOC // TT                   # 4 token tiles per core
KD = D // P                        # 16 k-subtiles for layer 1
M1 = H // P                        # 8 output tiles for layer 1
K2 = H // P                        # 8 k-subtiles for layer 2
M2 = H2 // P                       # 4 output tiles for layer 2
K3 = H2 // P                       # 4 k-subtiles for layer 3
CPT = TT // CHUNK                  # 4 chunks per token tile

_CACHE = {}


def _round_tf32(x: np.ndarray) -> np.ndarray:
    """fp32 -> tf32 (10 explicit mantissa bits) round-to-nearest-even.

    Pure uint32 arithmetic: u + (((u>>13)&1) + 0xFFF) can only wrap for
    inputs >= 0x...FF000 in the low bits of an all-ones exponent (NaN/Inf),
    which do not occur for the finite activations/weights here.
    """
    u = np.ascontiguousarray(x, dtype=np.float32).view(np.uint32)
    bias = ((u >> np.uint32(13)) & np.uint32(1)) + np.uint32(0xFFF)
    u = (u + bias) & np.uint32(0xFFFFE000)
    return u.view(np.float32)


def _build_module(passes=1):
    import concourse.bass as bass  # noqa: F401
    import concourse.mybir as mybir
    import concourse.tile as tile
    from concourse import bacc

    F32 = mybir.dt.float32
    F32R = mybir.dt.float32r
    ts = bass.ts

    nc = bacc.Bacc("TRN2", target_bir_lowering=False, debug=False)

    xt = nc.dram_tensor("xt", [NT, P, KD, TT], F32R, kind="ExternalInput")
    w1 = nc.dram_tensor("w1", [D, H], F32R, kind="ExternalInput")
    w2 = nc.dram_tensor("w2", [H, H2], F32R, kind="ExternalInput")
    w3 = nc.dram_tensor("w3", [H2, E], F32R, kind="ExternalInput")
    b1 = nc.dram_tensor("b1", [H], F32, kind="ExternalInput")
    b2 = nc.dram_tensor("b2", [H2], F32, kind="ExternalInput")
    csum_out = nc.dram_tensor("csum", [E, C_LOC], F32, kind="ExternalOutput")

    w1r = w1.rearrange("(kd p) h -> p kd h", p=P)
    w2r = w2.rearrange("(kh p) h -> p kh h", p=P)
    w3r = w3.rearrange("(kh p) e -> p kh e", p=P)
    b1r = b1.rearrange("(m p) -> p m", p=P)
    b2r = b2.rearrange("(m p) -> p m", p=P)

    Relu = mybir.ActivationFunctionType.Relu

    with tile.TileContext(nc) as tc:
        with (
            tc.tile_pool(name="wp", bufs=1) as wp,
            tc.tile_pool(name="xp", bufs=2) as xp,
            tc.tile_pool(name="hp", bufs=1) as hp,
            tc.tile_pool(name="op", bufs=1) as op,
            tc.tile_pool(name="ps", bufs=4, space="PSUM") as ps,
            tc.tile_pool(name="ps3", bufs=2, space="PSUM") as ps3,
        ):
            w1s = wp.tile([P, KD, H], F32R)
            w2s = wp.tile([P, K2, H2], F32R)
            w3s = wp.tile([P, K3, E], F32R)
            b1s = wp.tile([P, M1], F32)
            b2s = wp.tile([P, M2], F32)
            # w1 arrives as per-k slices so the first L1 k-loop can start as
            # soon as slice 0 + the first x tile land (pipelined startup)
            for k in range(KD):
                nc.sync.dma_start(w1s[:, k], w1r[:, k])
            nc.sync.dma_start(w2s[:], w2r)
            nc.sync.dma_start(w3s[:], w3r)
            nc.sync.dma_start(b1s[:], b1r)
            nc.sync.dma_start(b2s[:], b2r)

            csums = op.tile([E, C_LOC], F32)

            for _p in range(passes):
                for nt in range(NT):
                    x_t = xp.tile([P, KD, TT], F32R, tag="x")
                    for kc in range(0, KD, 4):
                        nc.sync.dma_start(x_t[:, kc : kc + 4], xt[nt, :, kc : kc + 4])

                    h1 = hp.tile([P, M1, TT], F32R, tag="h1")
                    for m in range(M1):
                        pt = ps.tile([P, TT], F32, tag="acc")
                        for k in range(KD):
                            nc.tensor.matmul(pt[:], w1s[:, k, ts(m, P)], x_t[:, k, :],
                                             start=(k == 0), stop=(k == KD - 1))
                        nc.scalar.activation(h1[:, m, :], pt[:], Relu, bias=b1s[:, m : m + 1])

                    h2 = hp.tile([P, M2, TT], F32R, tag="h2")
                    for m in range(M2):
                        pt = ps.tile([P, TT], F32, tag="acc")
                        for k in range(K2):
                            nc.tensor.matmul(pt[:], w2s[:, k, ts(m, P)], h1[:, k, :],
                                             start=(k == 0), stop=(k == K2 - 1))
                        nc.scalar.activation(h2[:, m, :], pt[:], Relu, bias=b2s[:, m : m + 1])

                    p3 = ps3.tile([E, TT], F32, tag="l3")
                    for k in range(K3):
                        nc.tensor.matmul(p3[:], w3s[:, k, :], h2[:, k, :],
                                         start=(k == 0), stop=(k == K3 - 1))
                    lg = op.tile([E, TT], F32, tag="lg")
                    nc.scalar.copy(lg[:], p3[:])
                    nc.vector.reduce_sum(csums[:, ts(nt, CPT)],
                                         lg.rearrange("p (c t) -> p c t", t=CHUNK),
                                         axis=mybir.AxisListType.X)

            nc.sync.dma_start(csum_out[:], csums[:])

    nc.compile()
    return nc


def _get_module():
    if "nc" not in _CACHE:
        _CACHE["nc"] = _build_module()
    return _CACHE["nc"]


def _device_chunk_sums(x, w1, w2, w3, b1, b2):
    """Run the sharded MLP on 8 NeuronCores; return chunk logit sums [B, C, E]
    (without the +b3 and /CHUNK, which the host applies)."""
    from concourse.bass_utils import run_bass_kernel_spmd

    nc = _get_module()

    w1r = _round_tf32(w1)
    w2r = _round_tf32(w2)
    w3r = _round_tf32(w3)
    b1c = np.ascontiguousarray(b1, dtype=np.float32)
    b2c = np.ascontiguousarray(b2, dtype=np.float32)

    xr = _round_tf32(x)                      # one contiguous pass over all of x
    flat = xr.reshape(NCORES, NT, TT, KD, P)
    in_maps = []
    for c in range(NCORES):
        # [T_LOC, D] -> [NT, P, KD, TT]: xt[nt, p, kd, tt] = x[nt*TT + tt, kd*P + p]
        in_maps.append({
            "xt": np.ascontiguousarray(flat[c].transpose(0, 3, 2, 1)),
            "w1": w1r, "w2": w2r, "w3": w3r, "b1": b1c, "b2": b2c,
        })

    res = run_bass_kernel_spmd(nc, in_maps, core_ids=list(range(NCORES)))
    csums = np.stack([r["csum"] for r in res.results])          # [8, E, C_LOC]
    return csums.transpose(0, 2, 1).reshape(B, S // CHUNK, E)   # [B, C, E]


def kernel(x, prev_expert_indices=None, w1=None, b1=None, w2=None, b2=None,
           w3=None, b3=None):
    x = np.ascontiguousarray(x, dtype=np.float32)
    C = S // CHUNK

    chunk_sums = _device_chunk_sums(x, w1, w2, w3, b1, b2)
    chunk_logits = (chunk_sums / np.float32(CHUNK)
                    + np.asarray(b3, np.float32)[None, None, :]).astype(np.float32)

    # Sequential hysteresis scan over chunks (mirrors the reference scan).
    top = np.argmax(chunk_logits, axis=-1).astype(np.int32)     # [B, C]
    prev = np.zeros(B, np.int32)
    experts = np.zeros((B, C), np.int32)
    flips = 0
    bidx = np.arange(B)
    for c in range(C):
        cur = chunk_logits[bidx, c, top[:, c]]
        pv = chunk_logits[bidx, c, prev]
        switch = (cur - pv) > np.float32(TAU)
        if c == 0:
            final = top[:, c]
        else:
            final = np.where(switch, top[:, c], prev)
            flips += int(switch.sum())
        experts[:, c] = final
        prev = final

    one_hot = np.zeros((B, C, E), np.float32)
    np.put_along_axis(one_hot, experts[..., None].astype(np.int64), 1.0, axis=2)
    routing_weights = np.broadcast_to(one_hot[:, :, None, :],
                                      (B, C, CHUNK, E)).reshape(B, S, E)
    routing_weights = np.ascontiguousarray(routing_weights)

    # Stats (fp32, mirroring the reference formulas)
    m = chunk_logits.max(-1, keepdims=True)
    ex = np.exp(chunk_logits - m, dtype=np.float32)
    probs = ex / ex.sum(-1, keepdims=True)
    gate_entropy = np.float32(-(probs * np.log(probs + np.float32(1e-8))).sum(-1).mean())
    expert_utilization = one_hot.mean((0, 1)).astype(np.float32)
    flip_rate = np.float32(flips) / np.float32(B * max(1, C - 1))
    routing_concentration = np.float32(np.linalg.norm(expert_utilization))

    return (routing_weights, experts, chunk_logits,
            np.float32(gate_entropy), expert_utilization,
            np.float32(flip_rate), np.float32(routing_concentration))
